# revision 20
# baseline (speedup 1.0000x reference)
"""Trainium2 Bass kernel for a 16-head attention block (B=2, S=2048, D=1024).

The reference discards its softmax, so attention reduces to
(Q K^T / sqrt(dk)) V = Q (K^T V) / sqrt(dk): per head only a 64x64 Gram
matrix G_h = K_h^T V_h is needed, never the SxS score matrix.

Sharding (tensor parallel over heads, data parallel over batch): each of the
8 cores owns one batch and 4 of the 16 heads — the matching 256-column slice
of w_q/w_k/w_v and 256-row slice of w_o — over the full 2048-token sequence.
Every core is fully independent (no device collective); each returns its
w_o partial product and the host sums the four head-group partials per batch
(+ b_o) while gathering, which is the unshard step for TP sharding.
"""

import sys

sys.path.insert(0, "/opt/trn_rl_repo")

import numpy as np
import ml_dtypes

import concourse.bacc as bacc
import concourse.tile as tile
import concourse.mybir as mybir
from concourse import bass_utils

B, S, D, H, DK = 2, 2048, 1024, 16, 64
NCORES = 8
HG = H // (NCORES // B)   # 4 heads per core
FH = HG * DK              # 256 head-features per core
NT = S // 128             # 16 sequence tiles
ND = D // 128             # 8 input-feature chunks
NPAIR = FH // 128         # 2 head pairs (2 heads = 128 features)
NSC = S // 512            # 4 sequence chunks of 512

DT = mybir.dt.bfloat16
NP_DT = ml_dtypes.bfloat16
F32 = mybir.dt.float32

_cache = {}


def _build():
    nc = bacc.Bacc("TRN2", target_bir_lowering=False, debug=False,
                   num_devices=NCORES)

    xqT = nc.dram_tensor("xqT", [D, S], DT, kind="ExternalInput")
    xkT = nc.dram_tensor("xkT", [D, S], DT, kind="ExternalInput")
    xvT = nc.dram_tensor("xvT", [D, S], DT, kind="ExternalInput")
    wqT = nc.dram_tensor("wqT", [D, FH], DT, kind="ExternalInput")
    wkT = nc.dram_tensor("wkT", [D, FH], DT, kind="ExternalInput")
    wvT = nc.dram_tensor("wvT", [D, FH], DT, kind="ExternalInput")
    woT = nc.dram_tensor("woT", [FH, D], DT, kind="ExternalInput")
    bk_rep = nc.dram_tensor("bk_rep", [128, FH], F32, kind="ExternalInput")
    bv_rep = nc.dram_tensor("bv_rep", [128, FH], F32, kind="ExternalInput")
    bqT = nc.dram_tensor("bqT", [128, NPAIR], F32, kind="ExternalInput")
    out_h = nc.dram_tensor("out", [S, D], DT, kind="ExternalOutput")

    add = mybir.AluOpType.add

    with tile.TileContext(nc) as tc:
        with (
            tc.tile_pool(name="sb", bufs=1) as sb,
            tc.tile_pool(name="ps", bufs=3, space="PSUM") as ps,
            tc.tile_pool(name="pso", bufs=3, space="PSUM") as pso,
        ):
            # --- PE warmup while the first DMAs stream in (HAM clock gate)
            warm_a = sb.tile([128, 128], DT, name="warm_a", tag="warm_a")
            warm_b = sb.tile([128, 512], DT, name="warm_b", tag="warm_b")
            nc.vector.memset(warm_a[:], 0.0)
            nc.vector.memset(warm_b[:], 0.0)
            for i in range(14):
                wp = ps.tile([128, 512], F32, name=f"wp{i}", tag="proj")
                nc.tensor.matmul(wp[:], warm_a[:], warm_b[:],
                                 start=True, stop=True)

            # --- SBUF allocations
            xk_sb = sb.tile([128, ND * S], DT, name="xk_sb", tag="xk_sb")
            xv_sb = sb.tile([128, ND * S], DT, name="xv_sb", tag="xv_sb")
            xq_sb = sb.tile([128, ND * S], DT, name="xq_sb", tag="xq_sb")
            wk_sb = sb.tile([128, ND * FH], DT, name="wk_sb", tag="wk_sb")
            wv_sb = sb.tile([128, ND * FH], DT, name="wv_sb", tag="wv_sb")
            wq_sb = sb.tile([128, ND * FH], DT, name="wq_sb", tag="wq_sb")
            wo_sb = sb.tile([128, NPAIR * D], DT, name="wo_sb", tag="wo_sb")
            bk_sb = sb.tile([128, FH], F32, name="bk_sb", tag="bk_sb")
            bv_sb = sb.tile([128, FH], F32, name="bv_sb", tag="bv_sb")
            bq_sb = sb.tile([128, NPAIR], F32, name="bq_sb", tag="bq_sb")
            K_sb = sb.tile([128, NT * FH], DT, name="K_sb", tag="K_sb")
            V_sb = sb.tile([128, NT * FH], DT, name="V_sb", tag="V_sb")
            QT_sb = sb.tile([128, NPAIR * S], DT, name="QT_sb", tag="QT_sb")
            G_sb = sb.tile([128, NPAIR * 128], F32, name="G_sb", tag="G_sb")
            Gbd = sb.tile([128, NPAIR * 128], DT, name="Gbd", tag="Gbd")
            AT_sb = sb.tile([128, NPAIR * S], DT, name="AT_sb", tag="AT_sb")

            # --- input DMAs. Activations (big) go on the sync HWDGE ring,
            # weights + biases (small) on the scalar HWDGE ring so the two
            # streams drain in parallel instead of FIFO on one ring.
            for d in range(ND):
                nc.sync.dma_start(out=xk_sb[:, d * S:(d + 1) * S],
                                  in_=xkT[d * 128:(d + 1) * 128, :])
                nc.sync.dma_start(out=wk_sb[:, d * FH:(d + 1) * FH],
                                  in_=wkT[d * 128:(d + 1) * 128, :])
            nc.sync.dma_start(out=bk_sb[:], in_=bk_rep[:, :])
            for d in range(ND):
                nc.sync.dma_start(out=xv_sb[:, d * S:(d + 1) * S],
                                  in_=xvT[d * 128:(d + 1) * 128, :])
                nc.sync.dma_start(out=wv_sb[:, d * FH:(d + 1) * FH],
                                  in_=wvT[d * 128:(d + 1) * 128, :])
            nc.sync.dma_start(out=bv_sb[:], in_=bv_rep[:, :])
            for d in range(ND):
                nc.sync.dma_start(out=xq_sb[:, d * S:(d + 1) * S],
                                  in_=xqT[d * 128:(d + 1) * 128, :])
                nc.sync.dma_start(out=wq_sb[:, d * FH:(d + 1) * FH],
                                  in_=wqT[d * 128:(d + 1) * 128, :])
            nc.sync.dma_start(out=bq_sb[:], in_=bqT[:, :])
            for a in range(NPAIR):
                nc.sync.dma_start(out=wo_sb[:, a * D:(a + 1) * D],
                                  in_=woT[a * 128:(a + 1) * 128, :])

            nc.vector.memset(G_sb[:], 0.0)

            # --- K / V projections into [s, head_feat] tiles [128, 256]
            def proj_natural(x_sb, w_sb, bias_sb, dst_sb, pfx):
                for t in range(NT):
                    p = ps.tile([128, FH], F32, name=f"{pfx}{t}", tag="proj")
                    for d in range(ND):
                        nc.tensor.matmul(
                            p[:],
                            x_sb[:, d * S + t * 128:d * S + (t + 1) * 128],
                            w_sb[:, d * FH:(d + 1) * FH],
                            start=(d == 0), stop=(d == ND - 1))
                    nc.vector.tensor_tensor(
                        out=dst_sb[:, t * FH:(t + 1) * FH],
                        in0=p[:, 0:FH], in1=bias_sb[:], op=add)

            proj_natural(xk_sb, wk_sb, bk_sb, K_sb, "pk")
            proj_natural(xv_sb, wv_sb, bv_sb, V_sb, "pv")

            # --- Gram matrices, two heads (128 feats) per accumulation chain
            for pr in range(NPAIR):
                pg = ps.tile([128, 128], F32, name=f"pg{pr}", tag="g", bufs=2)
                for t in range(NT):
                    nc.tensor.matmul(
                        pg[:],
                        K_sb[:, t * FH + pr * 128:t * FH + (pr + 1) * 128],
                        V_sb[:, t * FH + pr * 128:t * FH + (pr + 1) * 128],
                        start=(t == 0), stop=(t == NT - 1))
                # keep only the per-head diagonal blocks, scaled by 1/sqrt(dk)
                nc.vector.tensor_scalar_mul(
                    G_sb[0:64, pr * 128:pr * 128 + 64], pg[0:64, 0:64], 0.125)
                nc.vector.tensor_scalar_mul(
                    G_sb[64:128, pr * 128 + 64:(pr + 1) * 128],
                    pg[64:128, 64:128], 0.125)
            nc.vector.tensor_copy(out=Gbd[:], in_=G_sb[:])

            # --- partial output projection for one 512-token chunk
            def out_chunk(sc):
                for tt in range(NT // NSC):
                    t = sc * (NT // NSC) + tt
                    ot = sb.tile([128, D], DT, name=f"ot{t}", tag="out_t",
                                 bufs=4)
                    for o in range(D // 512):
                        po = pso.tile([128, 512], F32, name=f"po{t}{o}",
                                      tag="out")
                        for a in range(NPAIR):
                            nc.tensor.matmul(
                                po[:],
                                AT_sb[:, a * S + t * 128:a * S + t * 128 + 128],
                                wo_sb[:, a * D + o * 512:a * D + o * 512 + 512],
                                start=(a == 0), stop=(a == NPAIR - 1))
                        if (2 * t + o) % 2 == 0:
                            nc.vector.tensor_copy(
                                out=ot[:, o * 512:o * 512 + 512], in_=po[:])
                        else:
                            nc.scalar.copy(
                                out=ot[:, o * 512:o * 512 + 512], in_=po[:])
                    nc.sync.dma_start(out=out_h[t * 128:(t + 1) * 128, :],
                                      in_=ot[:])

            # --- per-512-token-chunk pipeline: Q proj -> A^T -> partial out,
            # with the output stage running one chunk behind so it hides the
            # A^T eviction latency (and the output DMA starts early)
            for sc in range(NSC):
                # Q projection into transposed layout Q^T[dq, s]
                for qb in range(NPAIR):
                    p = ps.tile([128, 512], F32, name=f"pq{qb}{sc}", tag="proj")
                    for d in range(ND):
                        nc.tensor.matmul(
                            p[:],
                            wq_sb[:, d * FH + qb * 128:d * FH + qb * 128 + 128],
                            xq_sb[:, d * S + sc * 512:d * S + sc * 512 + 512],
                            start=(d == 0), stop=(d == ND - 1))
                    nc.vector.tensor_scalar(
                        out=QT_sb[:, qb * S + sc * 512:qb * S + sc * 512 + 512],
                        in0=p[:], scalar1=bq_sb[:, qb:qb + 1], scalar2=None,
                        op0=add)

                # A^T = blockdiag(G/8)^T @ Q^T
                for pr in range(NPAIR):
                    pa = ps.tile([128, 512], F32, name=f"pa{pr}{sc}",
                                 tag="proj")
                    nc.tensor.matmul(
                        pa[:], Gbd[:, pr * 128:(pr + 1) * 128],
                        QT_sb[:, pr * S + sc * 512:pr * S + sc * 512 + 512],
                        start=True, stop=True)
                    if pr % 2 == 0:
                        nc.vector.tensor_copy(
                            out=AT_sb[:,
                                      pr * S + sc * 512:pr * S + sc * 512 + 512],
                            in_=pa[:])
                    else:
                        nc.scalar.copy(
                            out=AT_sb[:,
                                      pr * S + sc * 512:pr * S + sc * 512 + 512],
                            in_=pa[:])

                # output projection runs one chunk behind
                if sc > 0:
                    out_chunk(sc - 1)
            out_chunk(NSC - 1)

    nc.compile()
    return nc


def _prep_in_maps(q, k, v, w_q, b_q, w_k, b_k, w_v, b_v, w_o, b_o):
    q, k, v = (np.asarray(x, np.float32) for x in (q, k, v))
    wqT = np.ascontiguousarray(np.asarray(w_q, np.float32).T).astype(NP_DT)
    wkT = np.ascontiguousarray(np.asarray(w_k, np.float32).T).astype(NP_DT)
    wvT = np.ascontiguousarray(np.asarray(w_v, np.float32).T).astype(NP_DT)
    woT = np.ascontiguousarray(np.asarray(w_o, np.float32).T).astype(NP_DT)
    b_q32 = np.asarray(b_q, np.float32)
    b_k32 = np.asarray(b_k, np.float32)
    b_v32 = np.asarray(b_v, np.float32)

    xT = {}
    for b in range(B):
        xT[b] = (
            np.ascontiguousarray(q[b].T).astype(NP_DT),
            np.ascontiguousarray(k[b].T).astype(NP_DT),
            np.ascontiguousarray(v[b].T).astype(NP_DT),
        )

    in_maps = []
    for c in range(NCORES):
        b, hg = divmod(c, NCORES // B)
        F = slice(hg * FH, (hg + 1) * FH)
        qT_b, kT_b, vT_b = xT[b]
        in_maps.append({
            "xqT": qT_b, "xkT": kT_b, "xvT": vT_b,
            "wqT": np.ascontiguousarray(wqT[:, F]),
            "wkT": np.ascontiguousarray(wkT[:, F]),
            "wvT": np.ascontiguousarray(wvT[:, F]),
            "woT": np.ascontiguousarray(woT[F, :]),
            "bk_rep": np.ascontiguousarray(
                np.broadcast_to(b_k32[F], (128, FH))),
            "bv_rep": np.ascontiguousarray(
                np.broadcast_to(b_v32[F], (128, FH))),
            "bqT": np.ascontiguousarray(b_q32[F].reshape(NPAIR, 128).T),
        })
    return in_maps


def _run(in_maps, trace=False):
    if "nc" not in _cache:
        _cache["nc"] = _build()
    nc = _cache["nc"]
    last_err = None
    for _attempt in range(3):
        try:
            return bass_utils.run_bass_kernel_spmd(
                nc, in_maps, core_ids=list(range(NCORES)), trace=trace)
        except Exception as e:  # transient NRT failures happen under axon
            last_err = e
    raise last_err


def _assemble(res, b_o):
    ncg = NCORES // B
    out = np.empty((B, S, D), np.float32)
    for b in range(B):
        acc = res.results[b * ncg]["out"].astype(np.float32)
        for hg in range(1, ncg):
            acc += res.results[b * ncg + hg]["out"].astype(np.float32)
        acc += np.asarray(b_o, np.float32)[None, :]
        out[b] = acc
    return out


def kernel(q, k, v, w_q, b_q, w_k, b_k, w_v, b_v, w_o, b_o):
    in_maps = _prep_in_maps(q, k, v, w_q, b_q, w_k, b_k, w_v, b_v, w_o, b_o)
    res = _run(in_maps, trace=False)
    return _assemble(res, b_o)


def kernel_traced(q, k, v, w_q, b_q, w_k, b_k, w_v, b_v, w_o, b_o):
    """Same as kernel() but profiles on hardware; returns (out, exec_ns, res)."""
    in_maps = _prep_in_maps(q, k, v, w_q, b_q, w_k, b_k, w_v, b_v, w_o, b_o)
    res = _run(in_maps, trace=True)
    return _assemble(res, b_o), res.exec_time_ns, res


# revision 29
# speedup vs baseline: 1.1787x; 1.1787x over previous
"""Trainium2 Bass kernel for a 16-head attention block (B=2, S=2048, D=1024).

The reference discards its softmax, so attention reduces to
(Q K^T / sqrt(dk)) V = Q (K^T V) / sqrt(dk): per head only a 64x64 Gram
matrix G_h = K_h^T V_h is needed, never the SxS score matrix.

Sharding (tensor parallel over heads, data parallel over batch): each of the
8 cores owns one batch and 4 of the 16 heads — the matching 256-column slice
of w_q/w_k/w_v and 256-row slice of w_o — over the full 2048-token sequence.
Every core is fully independent (no device collective); each returns its
w_o partial product and the host sums the four head-group partials per batch
(+ b_o) while gathering, which is the unshard step for TP sharding.
"""

import sys

sys.path.insert(0, "/opt/trn_rl_repo")

import numpy as np
import ml_dtypes

import concourse.bacc as bacc
import concourse.tile as tile
import concourse.mybir as mybir
from concourse import bass_utils

B, S, D, H, DK = 2, 2048, 1024, 16, 64
NCORES = 8
HG = H // (NCORES // B)   # 4 heads per core
FH = HG * DK              # 256 head-features per core
NT = S // 128             # 16 sequence tiles
ND = D // 128             # 8 input-feature chunks
NPAIR = FH // 128         # 2 head pairs (2 heads = 128 features)
NSC = S // 512            # 4 sequence chunks of 512

DT = mybir.dt.bfloat16
NP_DT = ml_dtypes.bfloat16
F32 = mybir.dt.float32

_cache = {}


def _build():
    nc = bacc.Bacc("TRN2", target_bir_lowering=False, debug=False,
                   num_devices=NCORES)

    xqT = nc.dram_tensor("xqT", [D, S], DT, kind="ExternalInput")
    xkT = nc.dram_tensor("xkT", [D, S], DT, kind="ExternalInput")
    xvT = nc.dram_tensor("xvT", [D, S], DT, kind="ExternalInput")
    wqT = nc.dram_tensor("wqT", [D, FH], DT, kind="ExternalInput")
    wkT = nc.dram_tensor("wkT", [D, FH], DT, kind="ExternalInput")
    wvT = nc.dram_tensor("wvT", [D, FH], DT, kind="ExternalInput")
    woT = nc.dram_tensor("woT", [FH, D], DT, kind="ExternalInput")
    bk_rep = nc.dram_tensor("bk_rep", [128, FH], F32, kind="ExternalInput")
    bv_rep = nc.dram_tensor("bv_rep", [128, FH], F32, kind="ExternalInput")
    bqT = nc.dram_tensor("bqT", [128, NPAIR], F32, kind="ExternalInput")
    out_h = nc.dram_tensor("out", [S, D], DT, kind="ExternalOutput")

    add = mybir.AluOpType.add

    with tile.TileContext(nc) as tc:
        with (
            tc.tile_pool(name="sb", bufs=1) as sb,
            tc.tile_pool(name="ps", bufs=6, space="PSUM") as ps,
        ):
            # --- PE warmup while the first DMAs stream in (HAM clock gate)
            warm_a = sb.tile([128, 128], DT, name="warm_a", tag="warm_a")
            warm_b = sb.tile([128, 512], DT, name="warm_b", tag="warm_b")
            nc.vector.memset(warm_a[:], 0.0)
            nc.vector.memset(warm_b[:], 0.0)
            for i in range(8):
                wp = ps.tile([128, 512], F32, name=f"wp{i}", tag="proj")
                nc.tensor.matmul(wp[:], warm_a[:], warm_b[:],
                                 start=True, stop=True)

            # --- SBUF allocations
            xk_sb = sb.tile([128, ND * S], DT, name="xk_sb", tag="xk_sb")
            xv_sb = sb.tile([128, ND * S], DT, name="xv_sb", tag="xv_sb")
            xq_sb = sb.tile([128, ND * S], DT, name="xq_sb", tag="xq_sb")
            wk_sb = sb.tile([128, ND * FH], DT, name="wk_sb", tag="wk_sb")
            wv_sb = sb.tile([128, ND * FH], DT, name="wv_sb", tag="wv_sb")
            wq_sb = sb.tile([128, ND * FH], DT, name="wq_sb", tag="wq_sb")
            wo_sb = sb.tile([128, NPAIR * D], DT, name="wo_sb", tag="wo_sb")
            bk_sb = sb.tile([128, FH], F32, name="bk_sb", tag="bk_sb")
            bv_sb = sb.tile([128, FH], F32, name="bv_sb", tag="bv_sb")
            bq_sb = sb.tile([128, NPAIR], F32, name="bq_sb", tag="bq_sb")
            K_sb = sb.tile([128, NT * FH], DT, name="K_sb", tag="K_sb")
            V_sb = sb.tile([128, NT * FH], DT, name="V_sb", tag="V_sb")
            QT_sb = sb.tile([128, NPAIR * S], DT, name="QT_sb", tag="QT_sb")
            G_sb = sb.tile([128, NPAIR * 128], F32, name="G_sb", tag="G_sb")
            Gbd = sb.tile([128, NPAIR * 128], DT, name="Gbd", tag="Gbd")
            AT_sb = sb.tile([128, NPAIR * S], DT, name="AT_sb", tag="AT_sb")

            # --- input DMAs, in consumption order, all on the sync ring
            for d in range(ND):
                nc.sync.dma_start(out=xk_sb[:, d * S:(d + 1) * S],
                                  in_=xkT[d * 128:(d + 1) * 128, :])
                nc.sync.dma_start(out=wk_sb[:, d * FH:(d + 1) * FH],
                                  in_=wkT[d * 128:(d + 1) * 128, :])
            nc.sync.dma_start(out=bk_sb[:], in_=bk_rep[:, :])
            for d in range(ND):
                nc.sync.dma_start(out=xv_sb[:, d * S:(d + 1) * S],
                                  in_=xvT[d * 128:(d + 1) * 128, :])
                nc.sync.dma_start(out=wv_sb[:, d * FH:(d + 1) * FH],
                                  in_=wvT[d * 128:(d + 1) * 128, :])
            nc.sync.dma_start(out=bv_sb[:], in_=bv_rep[:, :])
            for d in range(ND):
                nc.sync.dma_start(out=xq_sb[:, d * S:(d + 1) * S],
                                  in_=xqT[d * 128:(d + 1) * 128, :])
                nc.sync.dma_start(out=wq_sb[:, d * FH:(d + 1) * FH],
                                  in_=wqT[d * 128:(d + 1) * 128, :])
            nc.sync.dma_start(out=bq_sb[:], in_=bqT[:, :])
            for a in range(NPAIR):
                nc.sync.dma_start(out=wo_sb[:, a * D:(a + 1) * D],
                                  in_=woT[a * 128:(a + 1) * 128, :])

            nc.vector.memset(G_sb[:], 0.0)

            # --- K / V projection into [s, head_feat] tiles [128, 256].
            # The first NCHAIN tiles keep their PSUM chains open and
            # accumulate d-outer so the PE works WHILE the x chunks arrive;
            # the remaining tiles run t-outer on fully-resident data.
            NCHAIN = 6

            def proj_evict(p, bias_sb, dst_sb, t):
                nc.vector.tensor_tensor(
                    out=dst_sb[:, t * FH:(t + 1) * FH],
                    in0=p[:, 0:FH], in1=bias_sb[:], op=add)

            def proj_natural(x_sb, w_sb, bias_sb, dst_sb, pfx, tile_hook=None):
                chains = [ps.tile([128, FH], F32, name=f"{pfx}c{t}",
                                  tag="proj") for t in range(NCHAIN)]
                for d in range(ND):
                    for t in range(NCHAIN):
                        nc.tensor.matmul(
                            chains[t][:],
                            x_sb[:, d * S + t * 128:d * S + (t + 1) * 128],
                            w_sb[:, d * FH:(d + 1) * FH],
                            start=(d == 0), stop=(d == ND - 1))
                for t in range(NCHAIN):
                    proj_evict(chains[t], bias_sb, dst_sb, t)
                for t in range(NCHAIN, NT):
                    p = ps.tile([128, FH], F32, name=f"{pfx}{t}", tag="proj")
                    for d in range(ND):
                        nc.tensor.matmul(
                            p[:],
                            x_sb[:, d * S + t * 128:d * S + (t + 1) * 128],
                            w_sb[:, d * FH:(d + 1) * FH],
                            start=(d == 0), stop=(d == ND - 1))
                    proj_evict(p, bias_sb, dst_sb, t)
                    if tile_hook is not None:
                        tile_hook(t)

            proj_natural(xk_sb, wk_sb, bk_sb, K_sb, "pk")

            # --- V projection with the Gram-matrix accumulation interleaved
            # (two heads = 128 feats per G chain, one PSUM bank each)
            pgs = [ps.tile([128, 128], F32, name=f"pg{pr}", tag=f"g{pr}",
                           bufs=1) for pr in range(NPAIR)]

            def g_tile(t):
                for pr in range(NPAIR):
                    nc.tensor.matmul(
                        pgs[pr][:],
                        K_sb[:, t * FH + pr * 128:t * FH + (pr + 1) * 128],
                        V_sb[:, t * FH + pr * 128:t * FH + (pr + 1) * 128],
                        start=(t == 0), stop=(t == NT - 1))

            gdone = [0]

            def v_hook(t):
                # stay one tile behind the V evictions
                while gdone[0] < t:
                    g_tile(gdone[0])
                    gdone[0] += 1

            proj_natural(xv_sb, wv_sb, bv_sb, V_sb, "pv", tile_hook=v_hook)
            while gdone[0] < NT:
                g_tile(gdone[0])
                gdone[0] += 1

            for pr in range(NPAIR):
                # keep only the per-head diagonal blocks, scaled by 1/sqrt(dk)
                nc.vector.tensor_scalar_mul(
                    G_sb[0:64, pr * 128:pr * 128 + 64],
                    pgs[pr][0:64, 0:64], 0.125)
                nc.vector.tensor_scalar_mul(
                    G_sb[64:128, pr * 128 + 64:(pr + 1) * 128],
                    pgs[pr][64:128, 64:128], 0.125)
            nc.vector.tensor_copy(out=Gbd[:], in_=G_sb[:])

            def q_evict(p, qb, sc):
                nc.vector.tensor_scalar(
                    out=QT_sb[:, qb * S + sc * 512:qb * S + sc * 512 + 512],
                    in0=p[:], scalar1=bq_sb[:, qb:qb + 1], scalar2=None,
                    op0=add)

            def at_chunk(sc):
                for pr in range(NPAIR):
                    pa = ps.tile([128, 512], F32, name=f"pa{pr}{sc}",
                                 tag="proj")
                    nc.tensor.matmul(
                        pa[:], Gbd[:, pr * 128:(pr + 1) * 128],
                        QT_sb[:, pr * S + sc * 512:pr * S + sc * 512 + 512],
                        start=True, stop=True)
                    if pr % 2 == 0:
                        nc.vector.tensor_copy(
                            out=AT_sb[:,
                                      pr * S + sc * 512:pr * S + sc * 512 + 512],
                            in_=pa[:])
                    else:
                        nc.scalar.copy(
                            out=AT_sb[:,
                                      pr * S + sc * 512:pr * S + sc * 512 + 512],
                            in_=pa[:])

            def out_chunk(sc):
                for tt in range(NT // NSC):
                    t = sc * (NT // NSC) + tt
                    ot = sb.tile([128, D], DT, name=f"ot{t}", tag="out_t",
                                 bufs=4)
                    for o in range(D // 512):
                        po = ps.tile([128, 512], F32, name=f"po{t}{o}",
                                     tag="proj")
                        for a in range(NPAIR):
                            nc.tensor.matmul(
                                po[:],
                                AT_sb[:, a * S + t * 128:a * S + t * 128 + 128],
                                wo_sb[:, a * D + o * 512:a * D + o * 512 + 512],
                                start=(a == 0), stop=(a == NPAIR - 1))
                        if (2 * t + o) % 2 == 0:
                            nc.vector.tensor_copy(
                                out=ot[:, o * 512:o * 512 + 512], in_=po[:])
                        else:
                            nc.scalar.copy(
                                out=ot[:, o * 512:o * 512 + 512], in_=po[:])
                    nc.sync.dma_start(out=out_h[t * 128:(t + 1) * 128, :],
                                      in_=ot[:])

            # --- Q projection: first two chunks accumulate d-outer (PE works
            # while the xq chunks stream in), then the per-chunk pipeline
            # Q -> A^T -> out runs one chunk behind.
            qchains = [ps.tile([128, 512], F32, name=f"pq{qb}{sc}",
                               tag="proj")
                       for sc in range(2) for qb in range(NPAIR)]
            for d in range(ND):
                for i, p in enumerate(qchains):
                    qb, sc = i % NPAIR, i // NPAIR
                    nc.tensor.matmul(
                        p[:],
                        wq_sb[:, d * FH + qb * 128:d * FH + qb * 128 + 128],
                        xq_sb[:, d * S + sc * 512:d * S + sc * 512 + 512],
                        start=(d == 0), stop=(d == ND - 1))
            for i, p in enumerate(qchains):
                qb, sc = i % NPAIR, i // NPAIR
                q_evict(p, qb, sc)
            at_chunk(0)
            at_chunk(1)
            for sc in range(2, NSC):
                for qb in range(NPAIR):
                    p = ps.tile([128, 512], F32, name=f"pq{qb}{sc}",
                                tag="proj")
                    for d in range(ND):
                        nc.tensor.matmul(
                            p[:],
                            wq_sb[:, d * FH + qb * 128:d * FH + qb * 128 + 128],
                            xq_sb[:, d * S + sc * 512:d * S + sc * 512 + 512],
                            start=(d == 0), stop=(d == ND - 1))
                    q_evict(p, qb, sc)
                at_chunk(sc)
                out_chunk(sc - 2)
            out_chunk(NSC - 2)
            out_chunk(NSC - 1)

    nc.compile()
    return nc


def _prep_in_maps(q, k, v, w_q, b_q, w_k, b_k, w_v, b_v, w_o, b_o):
    q, k, v = (np.asarray(x, np.float32) for x in (q, k, v))
    wqT = np.ascontiguousarray(np.asarray(w_q, np.float32).T).astype(NP_DT)
    wkT = np.ascontiguousarray(np.asarray(w_k, np.float32).T).astype(NP_DT)
    wvT = np.ascontiguousarray(np.asarray(w_v, np.float32).T).astype(NP_DT)
    woT = np.ascontiguousarray(np.asarray(w_o, np.float32).T).astype(NP_DT)
    b_q32 = np.asarray(b_q, np.float32)
    b_k32 = np.asarray(b_k, np.float32)
    b_v32 = np.asarray(b_v, np.float32)

    xT = {}
    for b in range(B):
        xT[b] = (
            np.ascontiguousarray(q[b].T).astype(NP_DT),
            np.ascontiguousarray(k[b].T).astype(NP_DT),
            np.ascontiguousarray(v[b].T).astype(NP_DT),
        )

    in_maps = []
    for c in range(NCORES):
        b, hg = divmod(c, NCORES // B)
        F = slice(hg * FH, (hg + 1) * FH)
        qT_b, kT_b, vT_b = xT[b]
        in_maps.append({
            "xqT": qT_b, "xkT": kT_b, "xvT": vT_b,
            "wqT": np.ascontiguousarray(wqT[:, F]),
            "wkT": np.ascontiguousarray(wkT[:, F]),
            "wvT": np.ascontiguousarray(wvT[:, F]),
            "woT": np.ascontiguousarray(woT[F, :]),
            "bk_rep": np.ascontiguousarray(
                np.broadcast_to(b_k32[F], (128, FH))),
            "bv_rep": np.ascontiguousarray(
                np.broadcast_to(b_v32[F], (128, FH))),
            "bqT": np.ascontiguousarray(b_q32[F].reshape(NPAIR, 128).T),
        })
    return in_maps


def _run(in_maps, trace=False):
    if "nc" not in _cache:
        _cache["nc"] = _build()
    nc = _cache["nc"]
    last_err = None
    for _attempt in range(3):
        try:
            return bass_utils.run_bass_kernel_spmd(
                nc, in_maps, core_ids=list(range(NCORES)), trace=trace)
        except Exception as e:  # transient NRT failures happen under axon
            last_err = e
    raise last_err


def _assemble(res, b_o):
    ncg = NCORES // B
    out = np.empty((B, S, D), np.float32)
    for b in range(B):
        acc = res.results[b * ncg]["out"].astype(np.float32)
        for hg in range(1, ncg):
            acc += res.results[b * ncg + hg]["out"].astype(np.float32)
        acc += np.asarray(b_o, np.float32)[None, :]
        out[b] = acc
    return out


def kernel(q, k, v, w_q, b_q, w_k, b_k, w_v, b_v, w_o, b_o):
    in_maps = _prep_in_maps(q, k, v, w_q, b_q, w_k, b_k, w_v, b_v, w_o, b_o)
    res = _run(in_maps, trace=False)
    return _assemble(res, b_o)


def kernel_traced(q, k, v, w_q, b_q, w_k, b_k, w_v, b_v, w_o, b_o):
    """Same as kernel() but profiles on hardware; returns (out, exec_ns, res)."""
    in_maps = _prep_in_maps(q, k, v, w_q, b_q, w_k, b_k, w_v, b_v, w_o, b_o)
    res = _run(in_maps, trace=True)
    return _assemble(res, b_o), res.exec_time_ns, res


# revision 31
# speedup vs baseline: 1.2310x; 1.0444x over previous
"""Trainium2 Bass kernel for a 16-head attention block (B=2, S=2048, D=1024).

The reference discards its softmax, so attention reduces to
(Q K^T / sqrt(dk)) V = Q (K^T V) / sqrt(dk): per head only a 64x64 Gram
matrix G_h = K_h^T V_h is needed, never the SxS score matrix.

Sharding (tensor parallel over heads, data parallel over batch): each of the
8 cores owns one batch and 4 of the 16 heads — the matching 256-column slice
of w_q/w_k/w_v and 256-row slice of w_o — over the full 2048-token sequence.
Every core is fully independent (no device collective); each returns its
w_o partial product and the host sums the four head-group partials per batch
(+ b_o) while gathering, which is the unshard step for TP sharding.
"""

import sys

sys.path.insert(0, "/opt/trn_rl_repo")

import numpy as np
import ml_dtypes

import concourse.bacc as bacc
import concourse.tile as tile
import concourse.mybir as mybir
from concourse import bass_utils

B, S, D, H, DK = 2, 2048, 1024, 16, 64
NCORES = 8
HG = H // (NCORES // B)   # 4 heads per core
FH = HG * DK              # 256 head-features per core
NT = S // 128             # 16 sequence tiles
ND = D // 128             # 8 input-feature chunks
NPAIR = FH // 128         # 2 head pairs (2 heads = 128 features)
NSC = S // 512            # 4 sequence chunks of 512

DT = mybir.dt.bfloat16
NP_DT = ml_dtypes.bfloat16
F32 = mybir.dt.float32

_cache = {}


def _build():
    nc = bacc.Bacc("TRN2", target_bir_lowering=False, debug=False,
                   num_devices=NCORES)

    xqT = nc.dram_tensor("xqT", [D, S], DT, kind="ExternalInput")
    xkT = nc.dram_tensor("xkT", [D, S], DT, kind="ExternalInput")
    xvT = nc.dram_tensor("xvT", [D, S], DT, kind="ExternalInput")
    wqT = nc.dram_tensor("wqT", [D, FH], DT, kind="ExternalInput")
    wkT = nc.dram_tensor("wkT", [D, FH], DT, kind="ExternalInput")
    wvT = nc.dram_tensor("wvT", [D, FH], DT, kind="ExternalInput")
    woT = nc.dram_tensor("woT", [FH, D], DT, kind="ExternalInput")
    bk_rep = nc.dram_tensor("bk_rep", [128, FH], F32, kind="ExternalInput")
    bv_rep = nc.dram_tensor("bv_rep", [128, FH], F32, kind="ExternalInput")
    bqT = nc.dram_tensor("bqT", [128, NPAIR], F32, kind="ExternalInput")
    out_h = nc.dram_tensor("out", [S, D], DT, kind="ExternalOutput")

    add = mybir.AluOpType.add

    with tile.TileContext(nc) as tc:
        with (
            tc.tile_pool(name="sb", bufs=1) as sb,
            tc.tile_pool(name="ps", bufs=6, space="PSUM") as ps,
        ):
            # --- PE warmup while the first DMAs stream in (HAM clock gate)
            warm_a = sb.tile([128, 128], DT, name="warm_a", tag="warm_a")
            warm_b = sb.tile([128, 512], DT, name="warm_b", tag="warm_b")
            nc.vector.memset(warm_a[:], 0.0)
            nc.vector.memset(warm_b[:], 0.0)
            for i in range(8):
                wp = ps.tile([128, 512], F32, name=f"wp{i}", tag="proj")
                nc.tensor.matmul(wp[:], warm_a[:], warm_b[:],
                                 start=True, stop=True)

            # --- SBUF allocations
            xk_sb = sb.tile([128, ND * S], DT, name="xk_sb", tag="xk_sb")
            xv_sb = sb.tile([128, ND * S], DT, name="xv_sb", tag="xv_sb")
            xq_sb = sb.tile([128, ND * S], DT, name="xq_sb", tag="xq_sb")
            wk_sb = sb.tile([128, ND * FH], DT, name="wk_sb", tag="wk_sb")
            wv_sb = sb.tile([128, ND * FH], DT, name="wv_sb", tag="wv_sb")
            wq_sb = sb.tile([128, ND * FH], DT, name="wq_sb", tag="wq_sb")
            wo_sb = sb.tile([128, NPAIR * D], DT, name="wo_sb", tag="wo_sb")
            bk_sb = sb.tile([128, FH], F32, name="bk_sb", tag="bk_sb")
            bv_sb = sb.tile([128, FH], F32, name="bv_sb", tag="bv_sb")
            bq_sb = sb.tile([128, NPAIR], F32, name="bq_sb", tag="bq_sb")
            K_sb = sb.tile([128, NT * FH], DT, name="K_sb", tag="K_sb")
            V_sb = sb.tile([128, NT * FH], DT, name="V_sb", tag="V_sb")
            QT_sb = sb.tile([128, NPAIR * S], DT, name="QT_sb", tag="QT_sb")
            G_sb = sb.tile([128, NPAIR * 128], F32, name="G_sb", tag="G_sb")
            Gbd = sb.tile([128, NPAIR * 128], DT, name="Gbd", tag="Gbd")
            AT_sb = sb.tile([128, NPAIR * S], DT, name="AT_sb", tag="AT_sb")

            # --- input DMAs, in consumption order, all on the sync ring
            for d in range(ND):
                nc.sync.dma_start(out=xk_sb[:, d * S:(d + 1) * S],
                                  in_=xkT[d * 128:(d + 1) * 128, :])
                nc.sync.dma_start(out=wk_sb[:, d * FH:(d + 1) * FH],
                                  in_=wkT[d * 128:(d + 1) * 128, :])
            nc.sync.dma_start(out=bk_sb[:], in_=bk_rep[:, :])
            for d in range(ND):
                nc.sync.dma_start(out=xv_sb[:, d * S:(d + 1) * S],
                                  in_=xvT[d * 128:(d + 1) * 128, :])
                nc.sync.dma_start(out=wv_sb[:, d * FH:(d + 1) * FH],
                                  in_=wvT[d * 128:(d + 1) * 128, :])
            nc.sync.dma_start(out=bv_sb[:], in_=bv_rep[:, :])
            for d in range(ND):
                nc.sync.dma_start(out=xq_sb[:, d * S:(d + 1) * S],
                                  in_=xqT[d * 128:(d + 1) * 128, :])
                nc.sync.dma_start(out=wq_sb[:, d * FH:(d + 1) * FH],
                                  in_=wqT[d * 128:(d + 1) * 128, :])
            nc.sync.dma_start(out=bq_sb[:], in_=bqT[:, :])
            for a in range(NPAIR):
                nc.sync.dma_start(out=wo_sb[:, a * D:(a + 1) * D],
                                  in_=woT[a * 128:(a + 1) * 128, :])

            nc.vector.memset(G_sb[:], 0.0)

            # --- K / V projection into [s, head_feat] tiles [128, 256].
            # The first NCHAIN tiles keep their PSUM chains open and
            # accumulate d-outer so the PE works WHILE the x chunks arrive;
            # the remaining tiles run t-outer on fully-resident data.
            NCHAIN = 6

            def proj_evict(p, bias_sb, dst_sb, t):
                nc.vector.tensor_tensor(
                    out=dst_sb[:, t * FH:(t + 1) * FH],
                    in0=p[:, 0:FH], in1=bias_sb[:], op=add)

            def proj_natural(x_sb, w_sb, bias_sb, dst_sb, pfx, tile_hook=None):
                chains = [ps.tile([128, FH], F32, name=f"{pfx}c{t}",
                                  tag="proj") for t in range(NCHAIN)]
                for d in range(ND):
                    for t in range(NCHAIN):
                        nc.tensor.matmul(
                            chains[t][:],
                            x_sb[:, d * S + t * 128:d * S + (t + 1) * 128],
                            w_sb[:, d * FH:(d + 1) * FH],
                            start=(d == 0), stop=(d == ND - 1))
                for t in range(NCHAIN):
                    proj_evict(chains[t], bias_sb, dst_sb, t)
                for t in range(NCHAIN, NT):
                    p = ps.tile([128, FH], F32, name=f"{pfx}{t}", tag="proj")
                    for d in range(ND):
                        nc.tensor.matmul(
                            p[:],
                            x_sb[:, d * S + t * 128:d * S + (t + 1) * 128],
                            w_sb[:, d * FH:(d + 1) * FH],
                            start=(d == 0), stop=(d == ND - 1))
                    proj_evict(p, bias_sb, dst_sb, t)
                    if tile_hook is not None:
                        tile_hook(t)

            proj_natural(xk_sb, wk_sb, bk_sb, K_sb, "pk")

            # --- V projection with the Gram-matrix accumulation interleaved
            # (two heads = 128 feats per G chain, one PSUM bank each)
            pgs = [ps.tile([128, 128], F32, name=f"pg{pr}", tag=f"g{pr}",
                           bufs=1) for pr in range(NPAIR)]

            def g_tile(t):
                for pr in range(NPAIR):
                    nc.tensor.matmul(
                        pgs[pr][:],
                        K_sb[:, t * FH + pr * 128:t * FH + (pr + 1) * 128],
                        V_sb[:, t * FH + pr * 128:t * FH + (pr + 1) * 128],
                        start=(t == 0), stop=(t == NT - 1))

            gdone = [0]

            def v_hook(t):
                # stay one tile behind the V evictions
                while gdone[0] < t:
                    g_tile(gdone[0])
                    gdone[0] += 1

            proj_natural(xv_sb, wv_sb, bv_sb, V_sb, "pv", tile_hook=v_hook)
            while gdone[0] < NT:
                g_tile(gdone[0])
                gdone[0] += 1

            for pr in range(NPAIR):
                # keep only the per-head diagonal blocks, scaled by 1/sqrt(dk)
                nc.vector.tensor_scalar_mul(
                    G_sb[0:64, pr * 128:pr * 128 + 64],
                    pgs[pr][0:64, 0:64], 0.125)
                nc.vector.tensor_scalar_mul(
                    G_sb[64:128, pr * 128 + 64:(pr + 1) * 128],
                    pgs[pr][64:128, 64:128], 0.125)
            nc.vector.tensor_copy(out=Gbd[:], in_=G_sb[:])

            def q_evict(p, qb, sc):
                dst = QT_sb[:, qb * S + sc * 512:qb * S + sc * 512 + 512]
                if qb % 2 == 0:
                    nc.vector.tensor_scalar(
                        out=dst, in0=p[:], scalar1=bq_sb[:, qb:qb + 1],
                        scalar2=None, op0=add)
                else:
                    nc.scalar.activation(
                        dst, p[:], mybir.ActivationFunctionType.Identity,
                        bias=bq_sb[:, qb:qb + 1])

            def at_chunk(sc):
                for pr in range(NPAIR):
                    pa = ps.tile([128, 512], F32, name=f"pa{pr}{sc}",
                                 tag="proj")
                    nc.tensor.matmul(
                        pa[:], Gbd[:, pr * 128:(pr + 1) * 128],
                        QT_sb[:, pr * S + sc * 512:pr * S + sc * 512 + 512],
                        start=True, stop=True)
                    if pr % 2 == 0:
                        nc.vector.tensor_copy(
                            out=AT_sb[:,
                                      pr * S + sc * 512:pr * S + sc * 512 + 512],
                            in_=pa[:])
                    else:
                        nc.scalar.copy(
                            out=AT_sb[:,
                                      pr * S + sc * 512:pr * S + sc * 512 + 512],
                            in_=pa[:])

            def out_chunk(sc):
                for tt in range(NT // NSC):
                    t = sc * (NT // NSC) + tt
                    ot = sb.tile([128, D], DT, name=f"ot{t}", tag="out_t",
                                 bufs=4)
                    for o in range(D // 512):
                        po = ps.tile([128, 512], F32, name=f"po{t}{o}",
                                     tag="proj")
                        for a in range(NPAIR):
                            nc.tensor.matmul(
                                po[:],
                                AT_sb[:, a * S + t * 128:a * S + t * 128 + 128],
                                wo_sb[:, a * D + o * 512:a * D + o * 512 + 512],
                                start=(a == 0), stop=(a == NPAIR - 1))
                        if (2 * t + o) % 2 == 0:
                            nc.vector.tensor_copy(
                                out=ot[:, o * 512:o * 512 + 512], in_=po[:])
                        else:
                            nc.scalar.copy(
                                out=ot[:, o * 512:o * 512 + 512], in_=po[:])
                    nc.sync.dma_start(out=out_h[t * 128:(t + 1) * 128, :],
                                      in_=ot[:])

            # --- Q projection: first two chunks accumulate d-outer (PE works
            # while the xq chunks stream in), then the per-chunk pipeline
            # Q -> A^T -> out runs one chunk behind.
            qchains = [ps.tile([128, 512], F32, name=f"pq{qb}{sc}",
                               tag="proj")
                       for sc in range(2) for qb in range(NPAIR)]
            for d in range(ND):
                for i, p in enumerate(qchains):
                    qb, sc = i % NPAIR, i // NPAIR
                    nc.tensor.matmul(
                        p[:],
                        wq_sb[:, d * FH + qb * 128:d * FH + qb * 128 + 128],
                        xq_sb[:, d * S + sc * 512:d * S + sc * 512 + 512],
                        start=(d == 0), stop=(d == ND - 1))
            for i, p in enumerate(qchains):
                qb, sc = i % NPAIR, i // NPAIR
                q_evict(p, qb, sc)
            at_chunk(0)
            at_chunk(1)
            for sc in range(2, NSC):
                for qb in range(NPAIR):
                    p = ps.tile([128, 512], F32, name=f"pq{qb}{sc}",
                                tag="proj")
                    for d in range(ND):
                        nc.tensor.matmul(
                            p[:],
                            wq_sb[:, d * FH + qb * 128:d * FH + qb * 128 + 128],
                            xq_sb[:, d * S + sc * 512:d * S + sc * 512 + 512],
                            start=(d == 0), stop=(d == ND - 1))
                    q_evict(p, qb, sc)
                at_chunk(sc)
                out_chunk(sc - 2)
            out_chunk(NSC - 2)
            out_chunk(NSC - 1)

    nc.compile()
    return nc


def _prep_in_maps(q, k, v, w_q, b_q, w_k, b_k, w_v, b_v, w_o, b_o):
    q, k, v = (np.asarray(x, np.float32) for x in (q, k, v))
    wqT = np.ascontiguousarray(np.asarray(w_q, np.float32).T).astype(NP_DT)
    wkT = np.ascontiguousarray(np.asarray(w_k, np.float32).T).astype(NP_DT)
    wvT = np.ascontiguousarray(np.asarray(w_v, np.float32).T).astype(NP_DT)
    woT = np.ascontiguousarray(np.asarray(w_o, np.float32).T).astype(NP_DT)
    b_q32 = np.asarray(b_q, np.float32)
    b_k32 = np.asarray(b_k, np.float32)
    b_v32 = np.asarray(b_v, np.float32)

    xT = {}
    for b in range(B):
        xT[b] = (
            np.ascontiguousarray(q[b].T).astype(NP_DT),
            np.ascontiguousarray(k[b].T).astype(NP_DT),
            np.ascontiguousarray(v[b].T).astype(NP_DT),
        )

    in_maps = []
    for c in range(NCORES):
        b, hg = divmod(c, NCORES // B)
        F = slice(hg * FH, (hg + 1) * FH)
        qT_b, kT_b, vT_b = xT[b]
        in_maps.append({
            "xqT": qT_b, "xkT": kT_b, "xvT": vT_b,
            "wqT": np.ascontiguousarray(wqT[:, F]),
            "wkT": np.ascontiguousarray(wkT[:, F]),
            "wvT": np.ascontiguousarray(wvT[:, F]),
            "woT": np.ascontiguousarray(woT[F, :]),
            "bk_rep": np.ascontiguousarray(
                np.broadcast_to(b_k32[F], (128, FH))),
            "bv_rep": np.ascontiguousarray(
                np.broadcast_to(b_v32[F], (128, FH))),
            "bqT": np.ascontiguousarray(b_q32[F].reshape(NPAIR, 128).T),
        })
    return in_maps


def _run(in_maps, trace=False):
    if "nc" not in _cache:
        _cache["nc"] = _build()
    nc = _cache["nc"]
    last_err = None
    for _attempt in range(3):
        try:
            return bass_utils.run_bass_kernel_spmd(
                nc, in_maps, core_ids=list(range(NCORES)), trace=trace)
        except Exception as e:  # transient NRT failures happen under axon
            last_err = e
    raise last_err


def _assemble(res, b_o):
    ncg = NCORES // B
    out = np.empty((B, S, D), np.float32)
    for b in range(B):
        acc = res.results[b * ncg]["out"].astype(np.float32)
        for hg in range(1, ncg):
            acc += res.results[b * ncg + hg]["out"].astype(np.float32)
        acc += np.asarray(b_o, np.float32)[None, :]
        out[b] = acc
    return out


def kernel(q, k, v, w_q, b_q, w_k, b_k, w_v, b_v, w_o, b_o):
    in_maps = _prep_in_maps(q, k, v, w_q, b_q, w_k, b_k, w_v, b_v, w_o, b_o)
    res = _run(in_maps, trace=False)
    return _assemble(res, b_o)


def kernel_traced(q, k, v, w_q, b_q, w_k, b_k, w_v, b_v, w_o, b_o):
    """Same as kernel() but profiles on hardware; returns (out, exec_ns, res)."""
    in_maps = _prep_in_maps(q, k, v, w_q, b_q, w_k, b_k, w_v, b_v, w_o, b_o)
    res = _run(in_maps, trace=True)
    return _assemble(res, b_o), res.exec_time_ns, res


# revision 33
# speedup vs baseline: 1.2569x; 1.0210x over previous
"""Trainium2 Bass kernel for a 16-head attention block (B=2, S=2048, D=1024).

The reference discards its softmax, so attention reduces to
(Q K^T / sqrt(dk)) V = Q (K^T V) / sqrt(dk): per head only a 64x64 Gram
matrix G_h = K_h^T V_h is needed, never the SxS score matrix.

Sharding (tensor parallel over heads, data parallel over batch): each of the
8 cores owns one batch and 4 of the 16 heads — the matching 256-column slice
of w_q/w_k/w_v and 256-row slice of w_o — over the full 2048-token sequence.
Every core is fully independent (no device collective); each returns its
w_o partial product and the host sums the four head-group partials per batch
(+ b_o) while gathering, which is the unshard step for TP sharding.
"""

import sys

sys.path.insert(0, "/opt/trn_rl_repo")

import numpy as np
import ml_dtypes

import concourse.bacc as bacc
import concourse.tile as tile
import concourse.mybir as mybir
from concourse import bass_utils

B, S, D, H, DK = 2, 2048, 1024, 16, 64
NCORES = 8
HG = H // (NCORES // B)   # 4 heads per core
FH = HG * DK              # 256 head-features per core
NT = S // 128             # 16 sequence tiles
ND = D // 128             # 8 input-feature chunks
NPAIR = FH // 128         # 2 head pairs (2 heads = 128 features)
NSC = S // 512            # 4 sequence chunks of 512

DT = mybir.dt.bfloat16
NP_DT = ml_dtypes.bfloat16
F32 = mybir.dt.float32

_cache = {}


def _build():
    nc = bacc.Bacc("TRN2", target_bir_lowering=False, debug=False,
                   num_devices=NCORES)

    xqT = nc.dram_tensor("xqT", [D, S], DT, kind="ExternalInput")
    xkT = nc.dram_tensor("xkT", [D, S], DT, kind="ExternalInput")
    xvT = nc.dram_tensor("xvT", [D, S], DT, kind="ExternalInput")
    wqT = nc.dram_tensor("wqT", [D, FH], DT, kind="ExternalInput")
    wkT = nc.dram_tensor("wkT", [D, FH], DT, kind="ExternalInput")
    wvT = nc.dram_tensor("wvT", [D, FH], DT, kind="ExternalInput")
    woT = nc.dram_tensor("woT", [FH, D], DT, kind="ExternalInput")
    bk_rep = nc.dram_tensor("bk_rep", [128, FH], F32, kind="ExternalInput")
    bv_rep = nc.dram_tensor("bv_rep", [128, FH], F32, kind="ExternalInput")
    bqT = nc.dram_tensor("bqT", [128, NPAIR], F32, kind="ExternalInput")
    out_h = nc.dram_tensor("out", [S, D], DT, kind="ExternalOutput")

    add = mybir.AluOpType.add

    with tile.TileContext(nc) as tc:
        with (
            tc.tile_pool(name="sb", bufs=1) as sb,
            tc.tile_pool(name="ps", bufs=6, space="PSUM") as ps,
        ):
            # --- PE warmup while the first DMAs stream in (HAM clock gate)
            warm_a = sb.tile([128, 128], DT, name="warm_a", tag="warm_a")
            warm_b = sb.tile([128, 512], DT, name="warm_b", tag="warm_b")
            nc.vector.memset(warm_a[:], 0.0)
            nc.vector.memset(warm_b[:], 0.0)
            for i in range(8):
                wp = ps.tile([128, 512], F32, name=f"wp{i}", tag="proj")
                nc.tensor.matmul(wp[:], warm_a[:], warm_b[:],
                                 start=True, stop=True)

            # --- SBUF allocations
            xk_sb = sb.tile([128, ND * S], DT, name="xk_sb", tag="xk_sb")
            xv_sb = sb.tile([128, ND * S], DT, name="xv_sb", tag="xv_sb")
            xq_sb = sb.tile([128, ND * S], DT, name="xq_sb", tag="xq_sb")
            wk_sb = sb.tile([128, ND * FH], DT, name="wk_sb", tag="wk_sb")
            wv_sb = sb.tile([128, ND * FH], DT, name="wv_sb", tag="wv_sb")
            wq_sb = sb.tile([128, ND * FH], DT, name="wq_sb", tag="wq_sb")
            wo_sb = sb.tile([128, NPAIR * D], DT, name="wo_sb", tag="wo_sb")
            bk_sb = sb.tile([128, FH], F32, name="bk_sb", tag="bk_sb")
            bv_sb = sb.tile([128, FH], F32, name="bv_sb", tag="bv_sb")
            bq_sb = sb.tile([128, NPAIR], F32, name="bq_sb", tag="bq_sb")
            K_sb = sb.tile([128, NT * FH], DT, name="K_sb", tag="K_sb")
            V_sb = sb.tile([128, NT * FH], DT, name="V_sb", tag="V_sb")
            QT_sb = sb.tile([128, NPAIR * S], DT, name="QT_sb", tag="QT_sb")
            G_sb = sb.tile([128, NPAIR * 128], F32, name="G_sb", tag="G_sb")
            Gbd = sb.tile([128, NPAIR * 128], DT, name="Gbd", tag="Gbd")
            wGO_sb = sb.tile([128, NPAIR * D], DT, name="wGO_sb",
                             tag="wGO_sb")

            # --- input DMAs, in consumption order, all on the sync ring
            for d in range(ND):
                nc.sync.dma_start(out=xk_sb[:, d * S:(d + 1) * S],
                                  in_=xkT[d * 128:(d + 1) * 128, :])
                nc.sync.dma_start(out=wk_sb[:, d * FH:(d + 1) * FH],
                                  in_=wkT[d * 128:(d + 1) * 128, :])
            nc.sync.dma_start(out=bk_sb[:], in_=bk_rep[:, :])
            for d in range(ND):
                nc.sync.dma_start(out=xv_sb[:, d * S:(d + 1) * S],
                                  in_=xvT[d * 128:(d + 1) * 128, :])
                nc.sync.dma_start(out=wv_sb[:, d * FH:(d + 1) * FH],
                                  in_=wvT[d * 128:(d + 1) * 128, :])
            nc.sync.dma_start(out=bv_sb[:], in_=bv_rep[:, :])
            for a in range(NPAIR):
                nc.sync.dma_start(out=wo_sb[:, a * D:(a + 1) * D],
                                  in_=woT[a * 128:(a + 1) * 128, :])
            for d in range(ND):
                nc.sync.dma_start(out=xq_sb[:, d * S:(d + 1) * S],
                                  in_=xqT[d * 128:(d + 1) * 128, :])
                nc.sync.dma_start(out=wq_sb[:, d * FH:(d + 1) * FH],
                                  in_=wqT[d * 128:(d + 1) * 128, :])
            nc.sync.dma_start(out=bq_sb[:], in_=bqT[:, :])

            nc.vector.memset(G_sb[:], 0.0)

            # --- K / V projection into [s, head_feat] tiles [128, 256].
            # The first NCHAIN tiles keep their PSUM chains open and
            # accumulate d-outer so the PE works WHILE the x chunks arrive;
            # the remaining tiles run t-outer on fully-resident data.
            NCHAIN = 6

            def proj_evict(p, bias_sb, dst_sb, t):
                nc.vector.tensor_tensor(
                    out=dst_sb[:, t * FH:(t + 1) * FH],
                    in0=p[:, 0:FH], in1=bias_sb[:], op=add)

            def proj_natural(x_sb, w_sb, bias_sb, dst_sb, pfx, tile_hook=None):
                chains = [ps.tile([128, FH], F32, name=f"{pfx}c{t}",
                                  tag="proj") for t in range(NCHAIN)]
                for d in range(ND):
                    for t in range(NCHAIN):
                        nc.tensor.matmul(
                            chains[t][:],
                            x_sb[:, d * S + t * 128:d * S + (t + 1) * 128],
                            w_sb[:, d * FH:(d + 1) * FH],
                            start=(d == 0), stop=(d == ND - 1))
                for t in range(NCHAIN):
                    proj_evict(chains[t], bias_sb, dst_sb, t)
                for t in range(NCHAIN, NT):
                    p = ps.tile([128, FH], F32, name=f"{pfx}{t}", tag="proj")
                    for d in range(ND):
                        nc.tensor.matmul(
                            p[:],
                            x_sb[:, d * S + t * 128:d * S + (t + 1) * 128],
                            w_sb[:, d * FH:(d + 1) * FH],
                            start=(d == 0), stop=(d == ND - 1))
                    proj_evict(p, bias_sb, dst_sb, t)
                    if tile_hook is not None:
                        tile_hook(t)

            proj_natural(xk_sb, wk_sb, bk_sb, K_sb, "pk")

            # --- V projection with the Gram-matrix accumulation interleaved
            # (two heads = 128 feats per G chain, one PSUM bank each)
            pgs = [ps.tile([128, 128], F32, name=f"pg{pr}", tag=f"g{pr}",
                           bufs=1) for pr in range(NPAIR)]

            def g_tile(t):
                # accumulates G' = V^T K, i.e. Gbd holds Gs^T blocks, which
                # is exactly the lhsT needed for wGO = Gs @ woT below
                for pr in range(NPAIR):
                    nc.tensor.matmul(
                        pgs[pr][:],
                        V_sb[:, t * FH + pr * 128:t * FH + (pr + 1) * 128],
                        K_sb[:, t * FH + pr * 128:t * FH + (pr + 1) * 128],
                        start=(t == 0), stop=(t == NT - 1))

            gdone = [0]

            def v_hook(t):
                # stay one tile behind the V evictions
                while gdone[0] < t:
                    g_tile(gdone[0])
                    gdone[0] += 1

            proj_natural(xv_sb, wv_sb, bv_sb, V_sb, "pv", tile_hook=v_hook)
            while gdone[0] < NT:
                g_tile(gdone[0])
                gdone[0] += 1

            for pr in range(NPAIR):
                # keep only the per-head diagonal blocks, scaled by 1/sqrt(dk)
                nc.vector.tensor_scalar_mul(
                    G_sb[0:64, pr * 128:pr * 128 + 64],
                    pgs[pr][0:64, 0:64], 0.125)
                nc.scalar.activation(
                    G_sb[64:128, pr * 128 + 64:(pr + 1) * 128],
                    pgs[pr][64:128, 64:128],
                    mybir.ActivationFunctionType.Identity, scale=0.125)
            nc.vector.tensor_copy(out=Gbd[:, 0:128], in_=G_sb[:, 0:128])
            nc.scalar.copy(out=Gbd[:, 128:256], in_=G_sb[:, 128:256])

            def q_evict(p, qb, sc):
                dst = QT_sb[:, qb * S + sc * 512:qb * S + sc * 512 + 512]
                if qb % 2 == 0:
                    nc.vector.tensor_scalar(
                        out=dst, in0=p[:], scalar1=bq_sb[:, qb:qb + 1],
                        scalar2=None, op0=add)
                else:
                    nc.scalar.activation(
                        dst, p[:], mybir.ActivationFunctionType.Identity,
                        bias=bq_sb[:, qb:qb + 1])

            def wgo_build():
                # wGO = Gs @ woT_slice, per head-pair block (block-diagonal
                # Gs means no accumulation across pairs)
                for ib in range(NPAIR):
                    for o in range(D // 512):
                        pw = ps.tile([128, 512], F32, name=f"pw{ib}{o}",
                                     tag="proj")
                        nc.tensor.matmul(
                            pw[:], Gbd[:, ib * 128:(ib + 1) * 128],
                            wo_sb[:, ib * D + o * 512:ib * D + o * 512 + 512],
                            start=True, stop=True)
                        dst = wGO_sb[:, ib * D + o * 512:ib * D + o * 512 + 512]
                        if (2 * ib + o) % 2 == 0:
                            nc.vector.tensor_copy(out=dst, in_=pw[:])
                        else:
                            nc.scalar.copy(out=dst, in_=pw[:])

            def out_chunk(sc):
                for tt in range(NT // NSC):
                    t = sc * (NT // NSC) + tt
                    ot = sb.tile([128, D], DT, name=f"ot{t}", tag="out_t",
                                 bufs=4)
                    for o in range(D // 512):
                        po = ps.tile([128, 512], F32, name=f"po{t}{o}",
                                     tag="proj")
                        for a in range(NPAIR):
                            nc.tensor.matmul(
                                po[:],
                                QT_sb[:, a * S + t * 128:a * S + t * 128 + 128],
                                wGO_sb[:, a * D + o * 512:a * D + o * 512 + 512],
                                start=(a == 0), stop=(a == NPAIR - 1))
                        if (2 * t + o) % 2 == 0:
                            nc.vector.tensor_copy(
                                out=ot[:, o * 512:o * 512 + 512], in_=po[:])
                        else:
                            nc.scalar.copy(
                                out=ot[:, o * 512:o * 512 + 512], in_=po[:])
                    nc.sync.dma_start(out=out_h[t * 128:(t + 1) * 128, :],
                                      in_=ot[:])

            # --- Q projection: first two chunks accumulate d-outer (PE works
            # while the xq chunks stream in); wGO is built first, then the
            # output stage runs one chunk behind the Q evictions.
            qchains = [ps.tile([128, 512], F32, name=f"pq{qb}{sc}",
                               tag="proj")
                       for sc in range(2) for qb in range(NPAIR)]
            for d in range(ND):
                for i, p in enumerate(qchains):
                    qb, sc = i % NPAIR, i // NPAIR
                    nc.tensor.matmul(
                        p[:],
                        wq_sb[:, d * FH + qb * 128:d * FH + qb * 128 + 128],
                        xq_sb[:, d * S + sc * 512:d * S + sc * 512 + 512],
                        start=(d == 0), stop=(d == ND - 1))
            wgo_build()
            for i, p in enumerate(qchains):
                qb, sc = i % NPAIR, i // NPAIR
                q_evict(p, qb, sc)
            for sc in range(2, NSC):
                for qb in range(NPAIR):
                    p = ps.tile([128, 512], F32, name=f"pq{qb}{sc}",
                                tag="proj")
                    for d in range(ND):
                        nc.tensor.matmul(
                            p[:],
                            wq_sb[:, d * FH + qb * 128:d * FH + qb * 128 + 128],
                            xq_sb[:, d * S + sc * 512:d * S + sc * 512 + 512],
                            start=(d == 0), stop=(d == ND - 1))
                    q_evict(p, qb, sc)
                out_chunk(sc - 2)
            out_chunk(NSC - 2)
            out_chunk(NSC - 1)

    nc.compile()
    return nc


def _prep_in_maps(q, k, v, w_q, b_q, w_k, b_k, w_v, b_v, w_o, b_o):
    q, k, v = (np.asarray(x, np.float32) for x in (q, k, v))
    wqT = np.ascontiguousarray(np.asarray(w_q, np.float32).T).astype(NP_DT)
    wkT = np.ascontiguousarray(np.asarray(w_k, np.float32).T).astype(NP_DT)
    wvT = np.ascontiguousarray(np.asarray(w_v, np.float32).T).astype(NP_DT)
    woT = np.ascontiguousarray(np.asarray(w_o, np.float32).T).astype(NP_DT)
    b_q32 = np.asarray(b_q, np.float32)
    b_k32 = np.asarray(b_k, np.float32)
    b_v32 = np.asarray(b_v, np.float32)

    xT = {}
    for b in range(B):
        xT[b] = (
            np.ascontiguousarray(q[b].T).astype(NP_DT),
            np.ascontiguousarray(k[b].T).astype(NP_DT),
            np.ascontiguousarray(v[b].T).astype(NP_DT),
        )

    in_maps = []
    for c in range(NCORES):
        b, hg = divmod(c, NCORES // B)
        F = slice(hg * FH, (hg + 1) * FH)
        qT_b, kT_b, vT_b = xT[b]
        in_maps.append({
            "xqT": qT_b, "xkT": kT_b, "xvT": vT_b,
            "wqT": np.ascontiguousarray(wqT[:, F]),
            "wkT": np.ascontiguousarray(wkT[:, F]),
            "wvT": np.ascontiguousarray(wvT[:, F]),
            "woT": np.ascontiguousarray(woT[F, :]),
            "bk_rep": np.ascontiguousarray(
                np.broadcast_to(b_k32[F], (128, FH))),
            "bv_rep": np.ascontiguousarray(
                np.broadcast_to(b_v32[F], (128, FH))),
            "bqT": np.ascontiguousarray(b_q32[F].reshape(NPAIR, 128).T),
        })
    return in_maps


def _run(in_maps, trace=False):
    if "nc" not in _cache:
        _cache["nc"] = _build()
    nc = _cache["nc"]
    last_err = None
    for _attempt in range(3):
        try:
            return bass_utils.run_bass_kernel_spmd(
                nc, in_maps, core_ids=list(range(NCORES)), trace=trace)
        except Exception as e:  # transient NRT failures happen under axon
            last_err = e
    raise last_err


def _assemble(res, b_o):
    ncg = NCORES // B
    out = np.empty((B, S, D), np.float32)
    for b in range(B):
        acc = res.results[b * ncg]["out"].astype(np.float32)
        for hg in range(1, ncg):
            acc += res.results[b * ncg + hg]["out"].astype(np.float32)
        acc += np.asarray(b_o, np.float32)[None, :]
        out[b] = acc
    return out


def kernel(q, k, v, w_q, b_q, w_k, b_k, w_v, b_v, w_o, b_o):
    in_maps = _prep_in_maps(q, k, v, w_q, b_q, w_k, b_k, w_v, b_v, w_o, b_o)
    res = _run(in_maps, trace=False)
    return _assemble(res, b_o)


def kernel_traced(q, k, v, w_q, b_q, w_k, b_k, w_v, b_v, w_o, b_o):
    """Same as kernel() but profiles on hardware; returns (out, exec_ns, res)."""
    in_maps = _prep_in_maps(q, k, v, w_q, b_q, w_k, b_k, w_v, b_v, w_o, b_o)
    res = _run(in_maps, trace=True)
    return _assemble(res, b_o), res.exec_time_ns, res
